# revision 1
# baseline (speedup 1.0000x reference)
"""4D conv (K0=3 outer taps x 3x3x3 inner, pad 1, stride 1) on 8 TRN2 cores.

Sharding: data-parallel over (batch, D0). 2 batches x 24 frames = 48 output
frames -> 6 per core. Each core receives its 8 input frames (6 + 2 halo,
zero-padded at the edges) directly from the host, so no device collectives.

Per-core kernel: direct convolution as a sum of 81 tap matmuls
(K=Ci=32 per tap).  PE packing:
  - 4x row tiling: 4 taps run concurrently in row groups (K=32 each),
    rhs read from 4 partition-replicated copies of the input tile.
  - 2x col tiling: two spatial chunks (N=507) run concurrently in column
    groups (M=64 output channels each).
Matmuls run in bf16 (host-converted inputs/weights) with fp32 PSUM
accumulation.  Row-group partials are summed on DVE (psum -> sbuf), with
the fp32 bias folded into the first reduction op.
"""

import ml_dtypes
import numpy as np

import concourse.bass as bass
import concourse.mybir as mybir
import concourse.tile as tile
from concourse.bass_utils import run_bass_kernel_spmd

F32 = mybir.dt.float32
BF16 = mybir.dt.bfloat16

# Problem constants (hardcoded per contract)
B, CI, O, D = 2, 32, 64, 24
K0 = 3
D2P = D + 2              # padded d2/d3 = 26
PLANE = D2P * D2P        # 676
D1P = D + 3              # d1 padded to 27: +1 conv pad each side, +1 OOB slack row
FRAMES_IN = 8            # 6 output frames + 2 halo input frames per core
FRAMES_OUT = 6
NSLAB = 4                # d1 slabs per frame (6 output rows each)
ROWS_OUT = 6             # output d1 rows per slab
NCHUNK = 507             # matmul moving free size ( = 6*676/8 )
HALF = 3 * PLANE         # 2028 = 3 output d1 rows, one col-group half
XTILE = (ROWS_OUT + 2) * PLANE + 56  # 8 input d1 rows + 56 OOB slack = 5464
NSLOT = 21               # ceil(81 taps / 4 row groups)
N_CORES = 8


def _build_nc(n_slabs=NSLAB, n_frames=FRAMES_OUT):
    nc = bass.Bass()
    x_h = nc.declare_dram_parameter("x", [CI, FRAMES_IN, D1P * PLANE], BF16, isOutput=False)
    w_h = nc.declare_dram_parameter("w", [128, NSLOT * 64], BF16, isOutput=False)
    b_h = nc.declare_dram_parameter("b", [128, 1], F32, isOutput=False)
    o_h = nc.declare_dram_parameter("out", [O, FRAMES_OUT, D * D * D], F32, isOutput=True)

    with tile.TileContext(nc) as tc:
        with (
            tc.tile_pool(name="wpool", bufs=1) as wpool,
            tc.tile_pool(name="xpool", bufs=4) as xpool,
            tc.tile_pool(name="opool", bufs=3) as opool,
            tc.tile_pool(name="psum", bufs=8, space="PSUM") as ppool,
        ):
            wt = wpool.tile([128, NSLOT * 64], BF16)
            nc.sync.dma_start(out=wt[:], in_=w_h[:])
            bt = wpool.tile([128, 1], F32)
            nc.sync.dma_start(out=bt[:], in_=b_h[:])

            for s in range(n_slabs):
                x_tiles = {}

                def get_x(fi, s=s, x_tiles=x_tiles):
                    if fi not in x_tiles:
                        t = xpool.tile([128, XTILE], BF16, tag="x")
                        src = x_h[:, fi,
                                  s * ROWS_OUT * PLANE: s * ROWS_OUT * PLANE + XTILE]
                        for g in range(4):
                            nc.sync.dma_start(out=t[32 * g: 32 * g + 32, :], in_=src)
                        x_tiles[fi] = t
                    return x_tiles[fi]

                for f in range(n_frames):
                    xs = [get_x(f + k0) for k0 in range(K0)]
                    osb = opool.tile([128, 4 * NCHUNK], F32, tag="osb")
                    # osb free layout: [pair*507 .. +507) per col half (=2028 total)
                    for pair in range(4):
                        ps = [
                            ppool.tile([128, NCHUNK], F32, tag="acc", name=f"acc{i}",
                                       padded_shape=[128, 512])
                            for i in range(4)
                        ]
                        for slot in range(NSLOT):
                            for g in range(4):
                                t_idx = slot * 4 + g
                                if t_idx > 80:
                                    continue
                                k0, kd1, kd2, kd3 = np.unravel_index(t_idx, (3, 3, 3, 3))
                                off = kd1 * PLANE + kd2 * D2P + kd3
                                lhs = wt[32 * g: 32 * g + 32,
                                         slot * 64: slot * 64 + 64]
                                xt = xs[k0]
                                last = (slot == NSLOT - 1) if g == 0 else (slot == NSLOT - 2)
                                for cg in range(2):
                                    base = cg * HALF + pair * NCHUNK + off
                                    rhs = xt[32 * g: 32 * g + 32,
                                             base: base + NCHUNK]
                                    nc.tensor.matmul(
                                        ps[g][64 * cg: 64 * cg + 64, :],
                                        lhs,
                                        rhs,
                                        start=(slot == 0),
                                        stop=last,
                                        tile_position=(32 * g, 64 * cg),
                                        # sim's zero-region group check drops
                                        # the partition base of sliced psum
                                        # outs; col groups use disjoint
                                        # partitions so groups are safe
                                        skip_group_check=True,
                                    )
                        # reduce 4 row-group partials into SBUF on DVE only
                        # (keeps psum-recycle WAR deps on a single DVE sem);
                        # bias folds into the first op as a per-partition scalar
                        osl = osb[:, pair * NCHUNK: (pair + 1) * NCHUNK]
                        nc.vector.tensor_scalar_add(osl, ps[0][:], bt[:])
                        for g in range(1, 4):
                            nc.vector.tensor_tensor(osl, osl, ps[g][:], mybir.AluOpType.add)
                    # write out both col halves: 3 valid d1 rows each
                    # (one DMA per d1 row: DMA APs only balance up to 3 dims)
                    for cg in range(2):
                        for r in range(3):
                            src = (
                                osb[64 * cg: 64 * cg + 64, :]
                                .rearrange("p (r c d) -> p r c d", r=3, c=D2P, d=D2P)
                                [:, r, :D, :D]
                            )
                            dst = (
                                o_h[:, f, :]
                                .rearrange("o (r c d) -> o r c d", r=D, c=D, d=D)
                                [:, s * ROWS_OUT + 3 * cg + r, :, :]
                            )
                            nc.sync.dma_start(out=dst, in_=src)
    return nc


# Instruction kinds whose waits live outside the engine sync-wait struct
# (DGE descriptors / barrier machinery) — leave those untouched.
_NO_SPLIT = {"EventSemaphore", "SemaphoreOp", "Call"}


def _split_multiwait(nc):
    """Walrus codegen accepts at most ONE sync wait per engine compute
    instruction on TRN2; Tile freely emits several. Hoist excess waits onto
    same-engine NoOps inserted just before the instruction — per-engine
    program order makes this equivalent."""
    ctr = 0
    for blk in nc.m.functions[0].blocks:
        il = blk.instructions
        i = 0
        while i < len(il):
            inst = il[i]
            si = inst.sync_info
            if (
                si is not None
                and len(si.on_wait) > 1
                and inst.opcode not in _NO_SPLIT
            ):
                waits = list(si.on_wait)
                nops = []
                for w in waits[:-1]:
                    ctr += 1
                    nop = mybir.InstNoOp(
                        name=f"I-wsplit-{ctr}", engine=inst.engine, ins=[], outs=[]
                    )
                    nop.sync_info = mybir.SyncInfo(on_wait=[w], on_update=[])
                    nops.append(nop)
                inst.sync_info = mybir.SyncInfo(
                    on_wait=[waits[-1]], on_update=list(si.on_update)
                )
                il[i:i] = nops
                i += len(nops)
            i += 1


_NC = None


def _get_nc():
    global _NC
    if _NC is None:
        _NC = _build_nc()
        _split_multiwait(_NC)
    return _NC


def _prep_inputs(x, w, b):
    """Host-side shard + pack. Returns list of per-core input maps."""
    x = np.asarray(x, dtype=np.float32)
    w = np.asarray(w, dtype=np.float32)
    b = np.asarray(b, dtype=np.float32)
    # pad: d0 by 1/1, d1 by 1/2 (extra OOB slack row), d2/d3 by 1/1
    xp = np.pad(x, ((0, 0), (0, 0), (1, 1), (1, 2), (1, 1), (1, 1)))
    # The reference's `xp.reshape(B*D0p, Ci, ...)` scrambles (B, Ci, D0p):
    # conv "frame" j has channels = flat volumes [j*32, j*32+32) of the
    # (B, Ci, D0p)-ordered volume pool. Output frame o of batch b sums tap i
    # applied to frame (b*26 + o + i).
    flat3 = xp.reshape(B * CI * (D + 2), D1P * PLANE)
    # w -> [tap(81), ci, o]
    arr = w.transpose(0, 3, 4, 5, 2, 1).reshape(81, CI, O)
    wsb = np.zeros((128, NSLOT * 64), dtype=ml_dtypes.bfloat16)
    for t in range(81):
        g, sl = t % 4, t // 4
        wsb[32 * g: 32 * g + 32, sl * 64: (sl + 1) * 64] = arr[t]
    bsb = np.ascontiguousarray(np.tile(b, 2).reshape(128, 1))
    in_maps = []
    for c in range(N_CORES):
        bi, fc = divmod(c, 4)
        j0 = bi * CI * (D + 2) + 32 * 6 * fc
        block = flat3[j0: j0 + 32 * FRAMES_IN]
        xc = np.ascontiguousarray(
            block.reshape(FRAMES_IN, CI, D1P * PLANE).transpose(1, 0, 2)
        ).astype(ml_dtypes.bfloat16)
        in_maps.append({"x": xc, "w": wsb, "b": bsb})
    return in_maps


def _assemble(results):
    out = np.empty((B, O, D, D, D, D), dtype=np.float32)
    for c in range(N_CORES):
        bi, fc = divmod(c, 4)
        r = results[c]["out"].reshape(O, FRAMES_OUT, D, D, D)
        out[bi, :, 6 * fc: 6 * fc + FRAMES_OUT] = r
    return out


def kernel(x, w, b):
    nc = _get_nc()
    in_maps = _prep_inputs(x, w, b)
    res = run_bass_kernel_spmd(nc, in_maps, list(range(N_CORES)))
    return _assemble(res.results)

